# revision 29
# baseline (speedup 1.0000x reference)
"""Cluster-local attention Trainium2 kernel (v2: fp8 DoubleRow).

Reference semantics:
    order = argsort(cluster_label, stable); xs = x[:, order]
    qkv = xs @ W_qkv + b_qkv ; q,k,v split, 8 heads x 64
    per (head, window of 256 sorted tokens): softmax(q k^T / 8) @ v
    h = attn_out @ W_out + b_out + xs            (returned in sorted order)

Sharding: 64 windows of 256 tokens -> 8 windows (2048 tokens) per core,
weights replicated, no collectives.

Numerics: the output is dominated by the fp32 residual (attention path is
~1.3% of |h|), so the attention path runs in fp8/bf16:
  - W_qkv, W_out host-scaled x32 and cast to fp8e4; x^T cast to fp8e4.
  - QKV / out projections use fp8 DoubleRow matmuls (2 k-tiles per pass).
  - scores S^T[k,q] in bf16 (64-deep contraction, no DR possible);
    exp on ACT -> fp8 probs; PV is one DoubleRow matmul per (head, qc)
    with a ones column at slot 64 giving the softmax row-sum.
  - normalize on DVE/Pool via reciprocal + broadcast tensor_tensor -> ao bf16
  - ao -> ao^T via DMA transpose (xbar); convert to fp8; out-projection in
    h^T form (fp8 DR); epilogue h^T = psum/1024 + x^T (fp32) -> DRAM.
Host side: pre-transpose x, cast weights; final h^T -> h transpose.
"""

import sys

if "/opt/trn_rl_repo" not in sys.path:
    sys.path.insert(0, "/opt/trn_rl_repo")

import os
import numpy as np
import ml_dtypes

B = 1
L = 16384
HID = 512
NH = 8
D = 64
WIN = 256
N_CORES = 8
T = L // N_CORES            # 2048 tokens per core
TT = T // 128               # 16 token tiles per core
NWIN = T // WIN             # 8 windows per core
VS = 72                     # per-head V slot: 64 v + 1 ones + 7 pad (alignment for fp8 DR)
WSCALE = 32.0               # host scale on W_qkv / W_out for fp8 range

_PROGRAM_CACHE = {}


def _build_program(has_bq: bool, has_bo: bool):
    import concourse.bacc as bacc
    import concourse.tile as tile
    import concourse.mybir as mybir

    fp32 = mybir.dt.float32
    bf16 = mybir.dt.bfloat16
    fp8 = mybir.dt.float8e4
    DR = mybir.MatmulPerfMode.DoubleRow

    nc = bacc.Bacc("TRN2", target_bir_lowering=False, debug=False,
                   enable_asserts=False, num_devices=N_CORES)

    xtf_d = nc.dram_tensor("xtf", [HID, T], fp32, kind="ExternalInput").ap()
    xt8_d = nc.dram_tensor("xt8", [HID, T], fp8, kind="ExternalInput").ap()
    w8_d = nc.dram_tensor("w8", [HID, 3 * HID], fp8, kind="ExternalInput").ap()
    wo8_d = nc.dram_tensor("wo8", [HID, HID], fp8, kind="ExternalInput").ap()
    if has_bq:
        bq_d = nc.dram_tensor("bq", [3 * HID], fp32, kind="ExternalInput").ap()
    if has_bo:
        bo_d = nc.dram_tensor("bo", [HID], fp32, kind="ExternalInput").ap()
    ht_d = nc.dram_tensor("ht", [HID, T], fp32, kind="ExternalOutput").ap()

    from contextlib import ExitStack

    with tile.TileContext(nc) as tc, ExitStack() as ctx:
        consts = ctx.enter_context(tc.tile_pool(name="consts", bufs=1))
        ex_p = ctx.enter_context(tc.tile_pool(name="exp", bufs=4))
        rc_p = ctx.enter_context(tc.tile_pool(name="rcp", bufs=4))
        proj_ps = ctx.enter_context(tc.tile_pool(
            name="proj_ps", bufs=2, space="PSUM"))
        st_ps = ctx.enter_context(tc.tile_pool(
            name="st_ps", bufs=2, space="PSUM"))
        pv_ps = ctx.enter_context(tc.tile_pool(
            name="pv_ps", bufs=2, space="PSUM"))

        # ---- persistent SBUF tensors -------------------------------------
        env = dict(
            xtf=consts.tile([128, 4 * T], fp32, name="xtf"),
            xt8=consts.tile([128, 4 * T], fp8, name="xt8"),
            w8=consts.tile([128, 4 * 3 * HID], fp8, name="w8"),
            wo8=consts.tile([128, 4 * HID], fp8, name="wo8"),
            qkt=consts.tile([128, 8 * T], bf16, name="qkt"),
            v65=consts.tile([128, TT * NH * VS], fp8, name="v65"),
            ao=consts.tile([128, TT * HID], bf16, name="ao"),
            aot=consts.tile([128, 4 * T], bf16, name="aot"),
            ao8=consts.tile([128, 4 * T], fp8, name="ao8"),
            hT=consts.tile([128, 4 * T], fp32, name="hT"),
            consts=consts, ex_p=ex_p, rc_p=rc_p,
            proj_ps=proj_ps, st_ps=st_ps, pv_ps=pv_ps,
            has_bq=has_bq, has_bo=has_bo,
            xtf_d=xtf_d, xt8_d=xt8_d, w8_d=w8_d, wo8_d=wo8_d, ht_d=ht_d,
        )
        if has_bq:
            env["bq_d"] = bq_d
        if has_bo:
            env["bo_d"] = bo_d

        repeat = int(os.environ.get("CLA_REPEAT", "1"))
        for _rep in range(repeat):
            _emit_body(nc, mybir, env)

    nc.compile()
    return nc


def _emit_body(nc, mybir, env):
    fp32 = mybir.dt.float32
    bf16 = mybir.dt.bfloat16
    fp8 = mybir.dt.float8e4
    AF = mybir.ActivationFunctionType
    DR = mybir.MatmulPerfMode.DoubleRow
    MUL = mybir.AluOpType.mult

    has_bq, has_bo = env["has_bq"], env["has_bo"]
    xtf, xt8, w8, wo8 = env["xtf"], env["xt8"], env["w8"], env["wo8"]
    qkt, v65, ao, aot, ao8, hT = (env["qkt"], env["v65"], env["ao"],
                                  env["aot"], env["ao8"], env["hT"])
    consts, ex_p, rc_p = env["consts"], env["ex_p"], env["rc_p"]
    proj_ps, st_ps, pv_ps = env["proj_ps"], env["st_ps"], env["pv_ps"]
    xtf_d, xt8_d, w8_d, wo8_d, ht_d = (env["xtf_d"], env["xt8_d"],
                                       env["w8_d"], env["wo8_d"],
                                       env["ht_d"])

    W3 = 3 * HID  # 1536

    # ---- loads: one 3D DMA per tensor (1 HWDGE ring slot each; a single
    # InstDMACopy is still split across all 16 SDMA engines)
    nc.sync.dma_start(
        out=xt8.rearrange("p (c t) -> p c t", t=T),
        in_=xt8_d.rearrange("(c p) t -> p c t", p=128))
    nc.scalar.dma_start(
        out=w8.rearrange("p (c x) -> p c x", x=W3),
        in_=w8_d.rearrange("(c p) x -> p c x", p=128))
    nc.sync.dma_start(
        out=wo8.rearrange("p (c x) -> p c x", x=HID),
        in_=wo8_d.rearrange("(c p) x -> p c x", p=128))
    nc.sync.dma_start(
        out=xtf.rearrange("p (c t) -> p c t", t=T),
        in_=xtf_d.rearrange("(c p) t -> p c t", p=128))

    if has_bq:
        bq_cols = consts.tile([128, 12], fp32)
        nc.sync.dma_start(out=bq_cols,
                          in_=env["bq_d"].rearrange("(m p) -> p m", p=128))
        ones_row = consts.tile([1, 128], fp8)
        nc.vector.memset(ones_row, 1.0)
        bqv_f = consts.tile([1, HID], fp32)
        nc.sync.dma_start(out=bqv_f,
                          in_=env["bq_d"][2 * HID:3 * HID].rearrange(
                              "(o d) -> o d", o=1))
        bqv_row = consts.tile([1, HID], fp8)
        nc.vector.tensor_copy(bqv_row, bqv_f)
    if has_bo:
        bo_cols = consts.tile([128, 4], fp32)
        nc.sync.dma_start(out=bo_cols,
                          in_=env["bo_d"].rearrange("(m p) -> p m", p=128))

    # ones column at 64, zero pad at 65:72 of every VS-col head slot
    nc.any.memset(
        v65.rearrange("p (s c) -> p s c", c=VS)[:, :, 64:65], 1.0)
    nc.any.memset(
        v65.rearrange("p (s c) -> p s c", c=VS)[:, :, 65:VS], 0.0)

    conv_pool = os.environ.get("CLA_CONV", "pool") == "pool"
    stage = int(os.environ.get("CLA_STAGE", "7"))

    def out_proj_half(n, half):
        if stage < 7:
            return
        for m in range(4):
            psf = proj_ps.tile([128, HID], fp32, tag="ps")
            ps = psf[:, 0:256]
            lo = n * 512 + half * 256
            for kp in range(2):
                for tt in range(2):
                    tg = lo // 128 + tt
                    nc.tensor.matmul(
                        ps[:, tt * 128:(tt + 1) * 128],
                        wo8.rearrange("p (kk x) -> p kk x", x=HID)[
                            :, 2 * kp:2 * kp + 2, m * 128:(m + 1) * 128],
                        ao8.rearrange("p (t c) -> p t c", c=512)[
                            :, tg, :].rearrange("p (kk x) -> p kk x", x=128)[
                            :, 2 * kp:2 * kp + 2, :],
                        start=(kp == 0 and tt == 0),
                        stop=(kp == 1 and tt == 1), perf_mode=DR)
            dst = hT[:, m * T + lo: m * T + lo + 256]
            nc.vector.scalar_tensor_tensor(
                out=dst, in0=ps, scalar=1.0 / (WSCALE * WSCALE),
                op0=MUL,
                in1=xtf[:, m * T + lo: m * T + lo + 256],
                op1=mybir.AluOpType.add)
            if has_bo:
                nc.vector.tensor_scalar_add(dst, dst, bo_cols[:, m:m + 1])
            if half == 1:
                nc.sync.dma_start(
                    out=ht_d[m * 128:(m + 1) * 128,
                             (n - 1) * 512:(n + 1) * 512],
                    in_=hT[:, m * T + (n - 1) * 512: m * T + (n + 1) * 512])

    def out_proj(n):
        if stage < 7:
            return
        # ---- out projection (h^T form) + residual epilogue, chunk n ------
        for m in range(4):
            ps = proj_ps.tile([128, HID], fp32, tag="ps")
            for kp in range(2):
                for tt in range(4):
                    tg = 4 * n + tt
                    nc.tensor.matmul(
                        ps[:, tt * 128:(tt + 1) * 128],
                        wo8.rearrange("p (kk x) -> p kk x", x=HID)[
                            :, 2 * kp:2 * kp + 2, m * 128:(m + 1) * 128],
                        ao8.rearrange("p (t c) -> p t c", c=512)[
                            :, tg, :].rearrange("p (kk x) -> p kk x", x=128)[
                            :, 2 * kp:2 * kp + 2, :],
                        start=(kp == 0 and tt == 0),
                        stop=(kp == 1 and tt == 3), perf_mode=DR)
            dst = hT[:, m * T + n * 512: m * T + (n + 1) * 512]
            nc.vector.scalar_tensor_tensor(
                out=dst, in0=ps, scalar=1.0 / (WSCALE * WSCALE),
                op0=MUL,
                in1=xtf[:, m * T + n * 512: m * T + (n + 1) * 512],
                op1=mybir.AluOpType.add)
            if has_bo:
                nc.vector.tensor_scalar_add(dst, dst, bo_cols[:, m:m + 1])
            if n % 2 == 1:
                nc.sync.dma_start(
                    out=ht_d[m * 128:(m + 1) * 128,
                             (n - 1) * 512:(n + 1) * 512],
                    in_=hT[:, m * T + (n - 1) * 512: m * T + (n + 1) * 512])

    # token-chunk-major emission so window w's inputs are produced early;
    # out_proj runs one chunk behind so PE never waits on transpose/convert
    cflip = 0
    carry = None
    for n in range(4):
        # ---- Q^T / K^T projection chunks (DoubleRow, W stationary) -------
        for m in range(8 if stage >= 2 else 0):
            ps = proj_ps.tile([128, HID], fp32, tag="ps")
            for kp in range(2):
                nc.tensor.matmul(
                    ps,
                    w8.rearrange("p (kk w) -> p kk w", w=W3)[
                        :, 2 * kp:2 * kp + 2, m * 128:(m + 1) * 128],
                    xt8.rearrange("p (kk t) -> p kk t", t=T)[
                        :, 2 * kp:2 * kp + 2, n * 512:(n + 1) * 512],
                    start=(kp == 0), stop=(kp == 1), perf_mode=DR)
            dst = qkt[:, m * T + n * 512: m * T + (n + 1) * 512]
            if cflip in (0, 2):
                if has_bq:
                    nc.vector.tensor_scalar_add(dst, ps, bq_cols[:, m:m + 1])
                else:
                    nc.vector.tensor_copy(dst, ps)
            else:
                if has_bq:
                    nc.scalar.activation(dst, ps, AF.Identity,
                                         bias=bq_cols[:, m:m + 1])
                else:
                    nc.scalar.activation(dst, ps, AF.Copy)
            cflip = (cflip + 1) % 5

        # ---- V projection token tiles 4n..4n+4 (DoubleRow, X stationary) -
        for t in range(4 * n, (4 * n + 4) if stage >= 3 else (4 * n)):
            ps = proj_ps.tile([128, HID], fp32, tag="ps")
            if has_bq:
                nc.tensor.matmul(ps, ones_row, bqv_row,
                                 start=True, stop=False)
            for kp in range(2):
                nc.tensor.matmul(
                    ps,
                    xt8.rearrange("p (kk t) -> p kk t", t=T)[
                        :, 2 * kp:2 * kp + 2, t * 128:(t + 1) * 128],
                    w8.rearrange("p (kk w) -> p kk w", w=W3)[
                        :, 2 * kp:2 * kp + 2, 2 * HID:3 * HID],
                    start=(kp == 0 and not has_bq), stop=(kp == 1),
                    perf_mode=DR)
            # strided copy scatters all 8 head slices into their 65-col slots
            nc.vector.tensor_copy(
                v65[:, t * NH * VS: (t + 1) * NH * VS]
                .rearrange("p (h c) -> p h c", c=VS)[:, :, 0:64],
                ps.rearrange("p (h c) -> p h c", c=64))

        # ---- attention for windows 2n, 2n+1 ------------------------------
        # (out_proj(n-1) slotted between the windows: PE fills exp gaps)
        att = int(os.environ.get("CLA_ATT", "5"))

        def scores_block(w, hp):
            st = st_ps.tile([128, 2 * 2 * WIN], fp32)
            for hi in range(2):
                hh = 2 * hp + hi
                mq = hh // 2
                mk = 4 + hh // 2
                prow = (hh % 2) * 64
                for kc in range(2):
                    nc.tensor.matmul(
                        st[:, hi * 2 * WIN + kc * WIN:
                           hi * 2 * WIN + (kc + 1) * WIN],
                        qkt[prow:prow + 64,
                            mk * T + w * WIN + kc * 128:
                            mk * T + w * WIN + (kc + 1) * 128],
                        qkt[prow:prow + 64,
                            mq * T + w * WIN: mq * T + (w + 1) * WIN],
                        start=True, stop=True)
            if att < 2:
                return None
            ex = ex_p.tile([128, 2 * 2 * WIN], fp8)
            nc.scalar.activation(ex, st, AF.Exp,
                                 scale=1.0 / (np.sqrt(D) * WSCALE ** 2))
            return ex

        def pv_block(w, hp, ex):
            if att < 3 or ex is None:
                return
            pv = pv_ps.tile([128, 4 * VS], fp32)
            for hi in range(2):
                hh = 2 * hp + hi
                for qc in range(2):
                    nc.tensor.matmul(
                        pv[:, (2 * hi + qc) * VS:(2 * hi + qc + 1) * VS],
                        ex.rearrange("p (h kc q) -> p h kc q", h=2, kc=2)[
                            :, hi, :, qc * 128:(qc + 1) * 128],
                        v65.rearrange("p (t s) -> p t s", s=NH * VS)[
                            :, 2 * w:2 * w + 2, hh * VS:(hh + 1) * VS],
                        start=True, stop=True, perf_mode=DR)
            if att < 4:
                return
            rc = rc_p.tile([128, 4], fp32)
            nc.vector.reciprocal(
                rc, pv.rearrange("p (s c) -> p s c", c=VS)[:, :, 64])
            if att < 5:
                return
            # ao[tok=2w+qc, head hh=2hp+hi] = pv[:, s=2hi+qc, 0:64]*rc[s]
            nc.vector.tensor_tensor(
                out=ao.rearrange("p (t x) -> p t x", x=HID)[
                    :, 2 * w:2 * w + 2,
                    2 * hp * D:(2 * hp + 2) * D].rearrange(
                        "p t (h d) -> p t h d", h=2),
                in0=pv.rearrange("p (h q c) -> p q h c", h=2, q=2)[
                    :, :, :, 0:64],
                in1=rc.rearrange("p (h q o) -> p q h o", h=2, o=1)
                    .broadcast_to([128, 2, 2, 64]),
                op=MUL)

        def post_window(w):
            # ao tiles 2w, 2w+1 complete: transpose + fp8-convert this
            # window's 256 token-columns; split out_proj(3) starts at w==6
            if stage >= 5:
                # tile-major aot layout (col = t*512 + c*128 + x): the
                # 2-tile window transpose output is contiguous -> 1 xbar
                # DMA per window instead of 2
                nc.scalar.dma_start(
                    out=aot.rearrange("p (c x) -> p c x", x=128)[
                        :, 8 * w:8 * w + 8, :],
                    in_=ao[:, w * 2 * HID:(w + 1) * 2 * HID],
                    transpose=True)
            if stage >= 6:
                conv_eng = nc.gpsimd if conv_pool else nc.vector
                conv_eng.tensor_copy(
                    ao8[:, w * 1024:(w + 1) * 1024],
                    aot[:, w * 1024:(w + 1) * 1024])
            if w == 6:
                out_proj_half(3, 0)

        def flush_pv(carry):
            pv_block(*carry)
            if carry[1] == 3:
                post_window(carry[0])

        def attn_window(w, carry):
            # emit scores(hp) then pv(hp-1): PV LDWEIGHTS hide under the
            # next head-pair's 256-col score streams
            for hp in range(4):
                ex = scores_block(w, hp)
                if carry is not None:
                    flush_pv(carry)
                carry = (w, hp, ex)
            return carry

        if stage >= 4:
            carry = attn_window(2 * n, carry)
        if n > 0:
            out_proj(n - 1)
        if stage >= 4:
            carry = attn_window(2 * n + 1, carry)


    if stage >= 4 and carry is not None:
        flush_pv(carry)
        carry = None
    out_proj_half(3, 1)


def _get_program(has_bq: bool, has_bo: bool):
    key = (has_bq, has_bo)
    if key not in _PROGRAM_CACHE:
        _PROGRAM_CACHE[key] = _build_program(has_bq, has_bo)
    return _PROGRAM_CACHE[key]


def make_in_maps(x, cluster_label, W_qkv, b_qkv, W_out, b_out):
    """Host-side prep: sort tokens, shard, transpose + cast. Returns
    (in_maps, has_bq, has_bo)."""
    x = np.asarray(x, dtype=np.float32).reshape(L, HID)
    labels = np.asarray(cluster_label)
    order = np.argsort(labels, kind="stable")
    if not np.array_equal(order, np.arange(L)):
        xs = np.ascontiguousarray(x[order])
    else:
        xs = x
    f8 = ml_dtypes.float8_e4m3
    w8 = (np.asarray(W_qkv, dtype=np.float32) * WSCALE).astype(f8)
    wo8 = (np.asarray(W_out, dtype=np.float32) * WSCALE).astype(f8)
    bq = np.asarray(b_qkv, dtype=np.float32).reshape(3 * HID)
    bo = np.asarray(b_out, dtype=np.float32).reshape(HID)
    has_bq = bool(np.any(bq != 0))
    has_bo = bool(np.any(bo != 0))

    in_maps = []
    for c in range(N_CORES):
        xtf = np.ascontiguousarray(xs[c * T:(c + 1) * T].T)
        m = {
            "xtf": xtf,
            "xt8": xtf.astype(f8),
            "w8": w8,
            "wo8": wo8,
        }
        if has_bq:
            m["bq"] = bq * WSCALE
        if has_bo:
            m["bo"] = bo
        in_maps.append(m)
    return in_maps, has_bq, has_bo


def kernel(x, cluster_label, W_qkv, b_qkv, W_out, b_out):
    from concourse.bass_utils import run_bass_kernel_spmd

    in_maps, has_bq, has_bo = make_in_maps(
        x, cluster_label, W_qkv, b_qkv, W_out, b_out)
    nc = _get_program(has_bq, has_bo)
    res = run_bass_kernel_spmd(nc, in_maps, list(range(N_CORES)), trace=False)
    h = np.concatenate(
        [np.asarray(res.results[c]["ht"]).T for c in range(N_CORES)], axis=0)
    return np.ascontiguousarray(h).reshape(B, L, HID).astype(np.float32)
